# revision 18
# baseline (speedup 1.0000x reference)
"""Distributed Trainium2 Bass kernel for the associative-embedding (AE) loss.

Problem: per image b (B=8), two tag maps (tm0 [J,256,256], tm1 [J,512,512]),
keypoints kps [NH, 3*J] (x, y, vis interleaved, NH=30 humans, J=17 joints).
Per level: gather tag values at (j, x, y), masked per-human mean, pull loss
(masked squared deviation / num_humans) + push loss (pairwise Gaussian of
means / num_humans^2).  Output: per-image loss [B] (sum over both levels).

Strategy: pure data-parallel over B across 8 NeuronCores (core b handles
image b).  The loss touches only the ~NH*2*J visible-keypoint elements of
the 22 MB of tag maps, so each core pulls exactly those scalars out of DRAM
via indirect (SWDGE) DMAs.  The HW indirect DMA emits one descriptor per
out-partition row (max 128 scattered elements per ~1.1 us instruction), so
the host packs ONLY the visible entries into ceil(V/128) chunks -- typically
5 instead of 8 for the full grid -- with the small remainder chunk LAST so
its data drains quickly after the final descriptor-generation burst.  Host-
baked one-hot matrices let the tensor engine reduce the chunk layout into
per-human sufficient statistics via lhsT = E*S, rhs = [L0, L1, S*L0, S*L1]
(the two DVE prep ops per chunk are mutually independent, so the last
chunk's critical path is one op deep); chunks are processed while later
gathers are still in flight.  The push loss uses a 32x32 DVE stream
transpose, Square-with-bias + one Exp on the scalar engine, a ones-vector
matmul, and a 2-op weighted reduce; all input-only quantities (masks,
reciprocal counts, 1/num_humans weights) are host-precomputed.  Per-core
output is one scalar; the host stacks the 8 scalars into the final [B]
vector.
"""

import numpy as np

B = 8
NH = 30
J = 17
H0 = W0 = 256
H1 = W1 = 512
N0 = J * H0 * W0
N1 = J * H1 * W1
NTOT = N0 + N1
BIG = 30.0                # pad rows -> exp(-(BIG+avg)^2/2) ~ 1e-170 ~ 0
                          # (kept small: ACT Square is a piecewise table and
                          # must stay accurate at BIG, unlike huge sentinels)

_CACHE = {}


# ---------------------------------------------------------------------------
# host-side input prep: valid-packed gather indices + one-hot reduction maps
# ---------------------------------------------------------------------------


def make_in_maps(tag_maps0, tag_maps1, kps0, kps1):
    tag_maps0 = np.asarray(tag_maps0, dtype=np.float32)
    tag_maps1 = np.asarray(tag_maps1, dtype=np.float32)
    kps0 = np.asarray(kps0, dtype=np.int64)
    kps1 = np.asarray(kps1, dtype=np.int64)
    jr = np.arange(J)[None, :]
    per_img = []
    nv_max = 0
    for b in range(B):
        xs0, ys0, vs0 = kps0[b, :, 0::3], kps0[b, :, 1::3], kps0[b, :, 2::3]
        xs1, ys1, vs1 = kps1[b, :, 0::3], kps1[b, :, 1::3], kps1[b, :, 2::3]
        idx_hlj = np.concatenate(
            [jr * (H0 * W0) + xs0 * W0 + ys0,
             N0 + jr * (H1 * W1) + xs1 * W1 + ys1], axis=1
        )  # [30, 34] flat index per (human, level*J+joint)
        mask = np.concatenate([vs0 != 0, vs1 != 0], axis=1)  # [30, 34] bool
        hh, cc = np.nonzero(mask)     # valid entries: human, level*J+joint
        per_img.append((idx_hlj, mask, hh, cc))
        nv_max = max(nv_max, len(hh))
    NC = -(-nv_max // 128)            # chunks of <=128 descriptors
    n_last = nv_max - 128 * (NC - 1)  # last (remainder) chunk size

    in_maps = []
    for b in range(B):
        idx_hlj, mask, hh, cc = per_img[b]
        nv = len(hh)
        idxc = np.zeros((128, NC), np.int32)
        E = np.zeros((128, NC * NH), np.float32)
        T = np.zeros((128, 4 * NC), np.float32)  # [L0, L1, 0, 0] per chunk
        r = np.arange(nv)
        ch, p = r // 128, r % 128
        lvl = (cc // J).astype(np.int64)
        idxc[p, ch] = idx_hlj[hh, cc]
        E[p, ch * NH + hh] = 1.0
        T[p, ch * 4 + lvl] = 1.0

        cnt = np.stack([mask[:, :J].sum(1), mask[:, J:].sum(1)], 1).astype(
            np.float32
        )
        has = (cnt > 0).astype(np.float32)
        rdh = has / np.maximum(cnt, 1.0)
        P = 1.0 / has.sum(0)          # [2] 1/num_humans per level
        # kf layout [128, 4*NC + NC*NH + 70]:
        #   cols 0:4NC            T/rhs region (host: L0,L1,0,0; DVE: S*L)
        #   cols 4NC:4NC+NC*NH    E one-hot
        #   next 2: -rdh | next 2: rdh | next 66 (row 0): w66
        kf = np.zeros((128, 4 * NC + NC * NH + 70), np.float32)
        kf[:, 0 : 4 * NC] = T
        kf[:, 4 * NC : 4 * NC + NC * NH] = E
        base = 4 * NC + NC * NH
        kf[0:NH, base : base + 2] = -rdh
        kf[0:NH, base + 2 : base + 4] = rdh
        w66 = np.concatenate(
            [np.full(32, P[0] ** 2), np.full(32, P[1] ** 2), [P[0], P[1]]]
        )
        kf[0, base + 4 : base + 70] = w66
        tm = np.concatenate(
            [tag_maps0[b].ravel(), tag_maps1[b].ravel()]
        ).reshape(NTOT, 1)
        in_maps.append({"tm": tm, "ki": idxc, "kf": kf})
    return in_maps, NC, n_last


# ---------------------------------------------------------------------------
# device kernel (raw Block bass: hand-placed semaphores, no TileContext)
# ---------------------------------------------------------------------------


def _build_nc(NC, n_last):
    from contextlib import ExitStack

    from concourse import bacc, mybir
    from concourse.bass import IndirectOffsetOnAxis

    f32 = mybir.dt.float32
    i32 = mybir.dt.int32
    Alu = mybir.AluOpType
    X = mybir.AxisListType.X
    Exp = mybir.ActivationFunctionType.Exp
    Square = mybir.ActivationFunctionType.Square
    KFW = 4 * NC + NC * NH + 70
    base = 4 * NC + NC * NH

    nc = bacc.Bacc()
    TM = nc.declare_dram_parameter("tm", [NTOT, 1], f32, isOutput=False)
    KI = nc.declare_dram_parameter("ki", [128, NC], i32, isOutput=False)
    KF = nc.declare_dram_parameter("kf", [128, KFW], f32, isOutput=False)
    OUT = nc.declare_dram_parameter("out", [1, 1], f32, isOutput=True)

    with ExitStack() as ctx:
        e = ctx.enter_context
        ki = e(nc.sbuf_tensor("ki_sb", [128, NC], i32))
        kf = e(nc.sbuf_tensor("kf_sb", [128, KFW], f32))
        S = e(nc.sbuf_tensor("S", [128, NC], f32))
        EST = e(nc.sbuf_tensor("EST", [128, NC * NH], f32))
        avg = e(nc.sbuf_tensor("avg", [NH, 2], f32))
        u = e(nc.sbuf_tensor("u", [NH, 2], f32))
        avgsrc = e(nc.sbuf_tensor("avgsrc", [32, 64], f32))
        avgT = e(nc.sbuf_tensor("avgT", [32, 64], f32))
        d2 = e(nc.sbuf_tensor("d2", [NH, 64], f32))
        pmZ = e(nc.sbuf_tensor("pmZ", [NH, 66], f32))  # [pm | pull]
        ones = e(nc.sbuf_tensor("ones", [NH, 1], f32))
        warm = e(nc.sbuf_tensor("warm", [1, 2], f32))
        fin = e(nc.sbuf_tensor("fin", [1, 66], f32))
        res = e(nc.sbuf_tensor("res", [1, 1], f32))
        ps_st = e(nc.psum_tensor("ps_st", [NH, 4], f32))
        ps_f = e(nc.psum_tensor("ps_f", [1, 66], f32))

        d_ki = e(nc.semaphore("d_ki"))
        d_kf = e(nc.semaphore("d_kf"))
        gs = [e(nc.semaphore(f"gs{c}")) for c in range(NC)]
        vdone = e(nc.semaphore("vdone"))
        adone = e(nc.semaphore("adone"))
        a_exp = e(nc.semaphore("a_exp"))
        p_st = e(nc.semaphore("p_st"))
        p_f = e(nc.semaphore("p_f"))
        d_out = e(nc.semaphore("d_out"))

        block = e(nc.Block())
        M = {}

        @block.vector
        def _(vector):
            n = 0

            def op(r, key=None):
                nonlocal n
                r.then_inc(vdone, 1)
                n += 1
                if key:
                    M[key] = n
                return n

            def wt(k):
                vector.wait_ge(vdone, k)

            # S zeroed so the partial last chunk's unwritten rows can't
            # inject NaNs through the 0-masked products
            op(vector.memset(S[:], 0.0), "sz")
            op(vector.memset(avgsrc[:], BIG))
            op(vector.memset(ones[:], 1.0))
            op(vector.memset(warm[:], 0.0), "warm")
            vector.wait_ge(d_kf, 16)
            # per chunk: rhs cols 2:4 = S*[L0,L1]; lhsT = E*S (independent)
            for c in range(NC):
                vector.wait_ge(gs[c], 16)
                op(vector.tensor_tensor(
                    out=kf[:, 4 * c + 2 : 4 * c + 4].rearrange(
                        "p (o l) -> p o l", o=1),
                    in0=S[:, c : c + 1].to_broadcast([128, 1, 2]),
                    in1=kf[:, 4 * c : 4 * c + 2].rearrange(
                        "p (o l) -> p o l", o=1),
                    op=Alu.mult))
                op(vector.tensor_tensor(
                    out=EST[:, c * NH : (c + 1) * NH].rearrange(
                        "p (o h) -> p o h", o=1),
                    in0=S[:, c : c + 1].to_broadcast([128, 1, NH]),
                    in1=kf[:, 4 * NC + c * NH : 4 * NC + (c + 1) * NH
                           ].rearrange("p (o h) -> p o h", o=1),
                    op=Alu.mult), f"T{c}")
            # stats landed in psum: bcast+transpose drive the push tail;
            # avg (the ACT bias) comes right after, off the transpose path
            vector.wait_ge(p_st, 1)
            a = op(vector.tensor_tensor(
                out=avgsrc[0:NH, :].rearrange("p (l j) -> p l j", l=2),
                in0=ps_st[:, 0:2].to_broadcast([NH, 2, 32]),
                in1=kf[0:NH, base : base + 2].to_broadcast([NH, 2, 32]),
                op=Alu.mult))
            wt(a)
            op(vector.transpose(avgT[:], avgsrc[:]))
            op(vector.tensor_tensor(
                out=avg[:], in0=ps_st[:, 0:2],
                in1=kf[0:NH, base + 2 : base + 4], op=Alu.mult), "avg")
            # pull stats while ACT runs the push exp
            wt(M["avg"])
            u_n = op(vector.tensor_tensor(
                out=u[:], in0=ps_st[:, 0:2], in1=avg[:], op=Alu.mult))
            wt(u_n)
            op(vector.tensor_tensor(
                out=pmZ[:, 64:66], in0=ps_st[:, 2:4], in1=u[:],
                op=Alu.subtract), "pull")
            # final weighted reduce of [pm | pull] colsums
            vector.wait_ge(p_f, 1)
            a = op(vector.tensor_tensor(
                out=fin[:], in0=ps_f[:], in1=kf[0:1, base + 4 : base + 70],
                op=Alu.mult))
            wt(a)
            op(vector.reduce_sum(out=res[:], in_=fin[:], axis=X), "res")

        @block.sync
        def _(sync):
            sync.dma_start(out=kf[:], in_=KF[:]).then_inc(d_kf, 16)
            sync.wait_ge(vdone, M["res"])
            sync.dma_start(out=OUT[:], in_=res[:]).then_inc(d_out, 16)
            sync.wait_ge(d_out, 16)

        @block.gpsimd
        def _(gpsimd):
            gpsimd.wait_ge(d_ki, 16)
            gpsimd.wait_ge(vdone, M["sz"])
            for c in range(NC):
                rows = 128 if c < NC - 1 else n_last
                gpsimd.indirect_dma_start(
                    out=S[0:rows, c : c + 1],
                    out_offset=None,
                    in_=TM[:],
                    in_offset=IndirectOffsetOnAxis(
                        ap=ki[0:rows, c : c + 1], axis=0
                    ),
                ).then_inc(gs[c], 16)

        @block.scalar
        def _(scalar):
            # ki fetch on the scalar HWDGE queue, in parallel with sync's kf
            scalar.dma_start(out=ki[:], in_=KI[:]).then_inc(d_ki, 16)
            scalar.wait_ge(vdone, M["warm"])
            scalar.activation(warm[:, 0:1], warm[:, 0:1], Exp).then_inc(
                adone, 1
            )
            scalar.activation(warm[:, 1:2], warm[:, 1:2], Square).then_inc(
                adone, 1
            )
            scalar.wait_ge(vdone, M["avg"])
            # d2[i, l*32+j] = (avg_j - avg_i)^2 ; avgT holds -avg_j, bias +avg_i
            scalar.activation(
                d2[:, 0:32], avgT[0:NH, 0:32], Square, bias=avg[:, 0:1]
            ).then_inc(adone, 1)
            scalar.activation(
                d2[:, 32:64], avgT[0:NH, 32:64], Square, bias=avg[:, 1:2]
            ).then_inc(adone, 1)
            scalar.wait_ge(adone, 4)
            scalar.activation(
                pmZ[:, 0:64], d2[:], Exp, scale=-0.5
            ).then_inc(a_exp, 1)

        @block.tensor
        def _(tensor):
            for c in range(NC):
                tensor.wait_ge(vdone, M[f"T{c}"])
                mm = tensor.matmul(
                    ps_st[:],
                    lhsT=EST[:, c * NH : (c + 1) * NH],
                    rhs=kf[:, 4 * c : 4 * c + 4],
                    start=(c == 0),
                    stop=(c == NC - 1),
                )
            mm.then_inc(p_st, 1)
            tensor.wait_ge(a_exp, 1)
            tensor.wait_ge(vdone, M["pull"])
            tensor.matmul(
                ps_f[:], lhsT=ones[:], rhs=pmZ[:], start=True, stop=True
            ).then_inc(p_f, 1)

    nc.finalize()
    return nc


def _get_nc(NC, n_last):
    key = (NC, n_last)
    if key not in _CACHE:
        _CACHE[key] = _build_nc(NC, n_last)
    return _CACHE[key]


def kernel(tag_maps0, tag_maps1, kps0, kps1):
    from concourse.bass_utils import run_bass_kernel_spmd

    in_maps, NC, n_last = make_in_maps(tag_maps0, tag_maps1, kps0, kps1)
    nc = _get_nc(NC, n_last)
    out = run_bass_kernel_spmd(nc, in_maps, core_ids=list(range(B)))
    return np.array(
        [np.asarray(out.results[b]["out"]).reshape(()) for b in range(B)],
        dtype=np.float32,
    )


# revision 27
# speedup vs baseline: 1.0076x; 1.0076x over previous
"""Distributed Trainium2 Bass kernel for the associative-embedding (AE) loss.

Problem: per image b (B=8), two tag maps (tm0 [J,256,256], tm1 [J,512,512]),
keypoints kps [NH, 3*J] (x, y, vis interleaved, NH=30 humans, J=17 joints).
Per level: gather tag values at (j, x, y), masked per-human mean, pull loss
(masked squared deviation / num_humans) + push loss (pairwise Gaussian of
means / num_humans^2).  Output: per-image loss [B] (sum over both levels).

Strategy: pure data-parallel over B across 8 NeuronCores (core b handles
image b).  The loss touches only the ~NH*2*J visible-keypoint elements of
the 22 MB of tag maps, so each core pulls exactly those scalars out of DRAM
via indirect (SWDGE) DMAs.  The HW indirect DMA emits one descriptor per
out-partition row (max 128 scattered elements per ~1.1 us instruction), so
the host packs ONLY the visible entries into ceil(V/128) chunks -- typically
5 instead of 8 for the full grid -- with the small remainder chunk LAST so
its data drains quickly after the final descriptor-generation burst.  Host-
baked one-hot matrices let the tensor engine reduce the chunk layout into
per-human sufficient statistics via lhsT = E*S, rhs = [L0, L1, S*L0, S*L1]
(the two DVE prep ops per chunk are mutually independent, so the last
chunk's critical path is one op deep); chunks are processed while later
gathers are still in flight.  The push loss uses a 32x32 DVE stream
transpose, Square-with-bias + one Exp on the scalar engine, a ones-vector
matmul, and a 2-op weighted reduce; all input-only quantities (masks,
reciprocal counts, 1/num_humans weights) are host-precomputed.  Per-core
output is one scalar; the host stacks the 8 scalars into the final [B]
vector.
"""

import numpy as np

B = 8
NH = 30
J = 17
H0 = W0 = 256
H1 = W1 = 512
N0 = J * H0 * W0
N1 = J * H1 * W1
NTOT = N0 + N1
BIG = 30.0                # pad rows -> exp(-(BIG+avg)^2/2) ~ 1e-170 ~ 0
                          # (kept small: ACT Square is a piecewise table and
                          # must stay accurate at BIG, unlike huge sentinels)

_CACHE = {}


# ---------------------------------------------------------------------------
# host-side input prep: valid-packed gather indices + one-hot reduction maps
# ---------------------------------------------------------------------------


def make_in_maps(tag_maps0, tag_maps1, kps0, kps1):
    tag_maps0 = np.asarray(tag_maps0, dtype=np.float32)
    tag_maps1 = np.asarray(tag_maps1, dtype=np.float32)
    kps0 = np.asarray(kps0, dtype=np.int64)
    kps1 = np.asarray(kps1, dtype=np.int64)
    jr = np.arange(J)[None, :]
    per_img = []
    nv_max = 0
    for b in range(B):
        xs0, ys0, vs0 = kps0[b, :, 0::3], kps0[b, :, 1::3], kps0[b, :, 2::3]
        xs1, ys1, vs1 = kps1[b, :, 0::3], kps1[b, :, 1::3], kps1[b, :, 2::3]
        idx_hlj = np.concatenate(
            [jr * (H0 * W0) + xs0 * W0 + ys0,
             N0 + jr * (H1 * W1) + xs1 * W1 + ys1], axis=1
        )  # [30, 34] flat index per (human, level*J+joint)
        mask = np.concatenate([vs0 != 0, vs1 != 0], axis=1)  # [30, 34] bool
        hh, cc = np.nonzero(mask)     # valid entries: human, level*J+joint
        per_img.append((idx_hlj, mask, hh, cc))
        nv_max = max(nv_max, len(hh))
    NC = -(-nv_max // 128)            # chunks of <=128 descriptors
    n_last = nv_max - 128 * (NC - 1)  # last (remainder) chunk size

    in_maps = []
    for b in range(B):
        idx_hlj, mask, hh, cc = per_img[b]
        nv = len(hh)
        idxc = np.zeros((128, NC), np.int32)
        E = np.zeros((128, NC * NH), np.float32)
        T = np.zeros((128, 4 * NC), np.float32)  # [L0, L1, 0, 0] per chunk
        r = np.arange(nv)
        ch, p = r // 128, r % 128
        lvl = (cc // J).astype(np.int64)
        idxc[p, ch] = idx_hlj[hh, cc]
        E[p, ch * NH + hh] = 1.0
        T[p, ch * 4 + lvl] = 1.0

        cnt = np.stack([mask[:, :J].sum(1), mask[:, J:].sum(1)], 1).astype(
            np.float32
        )
        has = (cnt > 0).astype(np.float32)
        rdh = has / np.maximum(cnt, 1.0)
        P = 1.0 / has.sum(0)          # [2] 1/num_humans per level
        # kf layout [128, 4*NC + NC*NH + 8]:
        #   cols 0:4NC            T/rhs region (host: L0,L1,0,0; DVE: S*L)
        #   cols 4NC:4NC+NC*NH    E one-hot
        #   next 2: -rdh | next 2: rdh | next 4 (row 0): w4
        kf = np.zeros((128, 4 * NC + NC * NH + 8), np.float32)
        kf[:, 0 : 4 * NC] = T
        kf[:, 4 * NC : 4 * NC + NC * NH] = E
        base = 4 * NC + NC * NH
        kf[0:NH, base : base + 2] = -rdh
        kf[0:NH, base + 2 : base + 4] = rdh
        kf[0, base + 4 : base + 8] = [P[0] ** 2, P[1] ** 2, P[0], P[1]]
        tm = np.concatenate(
            [tag_maps0[b].ravel(), tag_maps1[b].ravel()]
        ).reshape(NTOT, 1)
        in_maps.append({"tm": tm, "ki": idxc, "kf": kf})
    return in_maps, NC, n_last


# ---------------------------------------------------------------------------
# device kernel (raw Block bass: hand-placed semaphores, no TileContext)
# ---------------------------------------------------------------------------


def _build_nc(NC, n_last):
    from contextlib import ExitStack

    from concourse import bacc, mybir
    from concourse.bass import IndirectOffsetOnAxis

    f32 = mybir.dt.float32
    i32 = mybir.dt.int32
    Alu = mybir.AluOpType
    X = mybir.AxisListType.X
    Exp = mybir.ActivationFunctionType.Exp
    Square = mybir.ActivationFunctionType.Square
    KFW = 4 * NC + NC * NH + 8
    base = 4 * NC + NC * NH

    nc = bacc.Bacc()
    TM = nc.declare_dram_parameter("tm", [NTOT, 1], f32, isOutput=False)
    KI = nc.declare_dram_parameter("ki", [128, NC], i32, isOutput=False)
    KF = nc.declare_dram_parameter("kf", [128, KFW], f32, isOutput=False)
    OUT = nc.declare_dram_parameter("out", [1, 1], f32, isOutput=True)

    with ExitStack() as ctx:
        e = ctx.enter_context
        ki = e(nc.sbuf_tensor("ki_sb", [128, NC], i32))
        kf = e(nc.sbuf_tensor("kf_sb", [128, KFW], f32))
        S = e(nc.sbuf_tensor("S", [128, NC], f32))
        EST = e(nc.sbuf_tensor("EST", [128, NC * NH], f32))
        avg = e(nc.sbuf_tensor("avg", [NH, 2], f32))
        u = e(nc.sbuf_tensor("u", [NH, 2], f32))
        avgsrc = e(nc.sbuf_tensor("avgsrc", [32, 64], f32))
        avgT = e(nc.sbuf_tensor("avgT", [32, 64], f32))
        d2 = e(nc.sbuf_tensor("d2", [NH, 64], f32))
        pm = e(nc.sbuf_tensor("pm", [NH, 64], f32))
        Z = e(nc.sbuf_tensor("Z", [NH, 4], f32))
        ones = e(nc.sbuf_tensor("ones", [NH, 1], f32))
        warm = e(nc.sbuf_tensor("warm", [1, 2], f32))
        fin = e(nc.sbuf_tensor("fin", [1, 4], f32))
        res = e(nc.sbuf_tensor("res", [1, 1], f32))
        ps_st = e(nc.psum_tensor("ps_st", [NH, 4], f32))
        ps_f = e(nc.psum_tensor("ps_f", [1, 4], f32))

        d_ki = e(nc.semaphore("d_ki"))
        d_kf = e(nc.semaphore("d_kf"))
        gs = [e(nc.semaphore(f"gs{c}")) for c in range(NC)]
        vdone = e(nc.semaphore("vdone"))
        adone = e(nc.semaphore("adone"))
        a_exp = e(nc.semaphore("a_exp"))
        p_st = e(nc.semaphore("p_st"))
        p_f = e(nc.semaphore("p_f"))
        d_out = e(nc.semaphore("d_out"))

        block = e(nc.Block())
        M = {}

        @block.vector
        def _(vector):
            n = 0

            def op(r, key=None):
                nonlocal n
                r.then_inc(vdone, 1)
                n += 1
                if key:
                    M[key] = n
                return n

            def wt(k):
                vector.wait_ge(vdone, k)

            # S zeroed so the partial last chunk's unwritten rows can't
            # inject NaNs through the 0-masked products
            op(vector.memset(S[:], 0.0), "sz")
            op(vector.memset(avgsrc[:], BIG))
            op(vector.memset(ones[:], 1.0))
            op(vector.memset(warm[:], 0.0), "warm")
            vector.wait_ge(d_kf, 16)
            # per chunk: rhs cols 2:4 = S*[L0,L1]; lhsT = E*S (independent)
            for c in range(NC):
                vector.wait_ge(gs[c], 16)
                op(vector.tensor_tensor(
                    out=kf[:, 4 * c + 2 : 4 * c + 4].rearrange(
                        "p (o l) -> p o l", o=1),
                    in0=S[:, c : c + 1].to_broadcast([128, 1, 2]),
                    in1=kf[:, 4 * c : 4 * c + 2].rearrange(
                        "p (o l) -> p o l", o=1),
                    op=Alu.mult))
                op(vector.tensor_tensor(
                    out=EST[:, c * NH : (c + 1) * NH].rearrange(
                        "p (o h) -> p o h", o=1),
                    in0=S[:, c : c + 1].to_broadcast([128, 1, NH]),
                    in1=kf[:, 4 * NC + c * NH : 4 * NC + (c + 1) * NH
                           ].rearrange("p (o h) -> p o h", o=1),
                    op=Alu.mult), f"T{c}")
            # stats landed in psum: bcast+transpose drive the push tail;
            # avg (the ACT bias) comes right after, off the transpose path
            vector.wait_ge(p_st, 1)
            a = op(vector.tensor_tensor(
                out=avgsrc[0:NH, :].rearrange("p (l j) -> p l j", l=2),
                in0=ps_st[:, 0:2].to_broadcast([NH, 2, 32]),
                in1=kf[0:NH, base : base + 2].to_broadcast([NH, 2, 32]),
                op=Alu.mult))
            wt(a)
            op(vector.transpose(avgT[:], avgsrc[:]))
            op(vector.tensor_tensor(
                out=avg[:], in0=ps_st[:, 0:2],
                in1=kf[0:NH, base + 2 : base + 4], op=Alu.mult), "avg")
            # pull stats while ACT runs the push exp
            wt(M["avg"])
            u_n = op(vector.tensor_tensor(
                out=u[:], in0=ps_st[:, 0:2], in1=avg[:], op=Alu.mult))
            wt(u_n)
            op(vector.tensor_tensor(
                out=Z[:, 2:4], in0=ps_st[:, 2:4], in1=u[:],
                op=Alu.subtract), "pull")
            # push row block sums once ACT finished the exp
            vector.wait_ge(a_exp, 1)
            op(vector.reduce_sum(
                out=Z[:, 0:2],
                in_=pm[:].rearrange("p (l j) -> p l j", l=2), axis=X),
                "push")
            # final weighted reduce of [push0, push1, pull0, pull1]
            vector.wait_ge(p_f, 1)
            a = op(vector.tensor_tensor(
                out=fin[:], in0=ps_f[:], in1=kf[0:1, base + 4 : base + 8],
                op=Alu.mult))
            wt(a)
            op(vector.reduce_sum(out=res[:], in_=fin[:], axis=X), "res")

        @block.sync
        def _(sync):
            sync.dma_start(out=ki[:], in_=KI[:]).then_inc(d_ki, 16)
            sync.dma_start(out=kf[:], in_=KF[:]).then_inc(d_kf, 16)
            sync.wait_ge(vdone, M["res"])
            sync.dma_start(out=OUT[:], in_=res[:]).then_inc(d_out, 16)
            sync.wait_ge(d_out, 16)

        @block.gpsimd
        def _(gpsimd):
            gpsimd.wait_ge(d_ki, 16)
            gpsimd.wait_ge(vdone, M["sz"])
            for c in range(NC):
                rows = 128 if c < NC - 1 else n_last
                gpsimd.indirect_dma_start(
                    out=S[0:rows, c : c + 1],
                    out_offset=None,
                    in_=TM[:],
                    in_offset=IndirectOffsetOnAxis(
                        ap=ki[0:rows, c : c + 1], axis=0
                    ),
                ).then_inc(gs[c], 16)

        @block.scalar
        def _(scalar):
            scalar.wait_ge(vdone, M["warm"])
            scalar.activation(warm[:, 0:1], warm[:, 0:1], Exp).then_inc(
                adone, 1
            )
            scalar.activation(warm[:, 1:2], warm[:, 1:2], Square).then_inc(
                adone, 1
            )
            scalar.wait_ge(vdone, M["avg"])
            # d2[i, l*32+j] = (avg_j - avg_i)^2 ; avgT holds -avg_j, bias +avg_i
            scalar.activation(
                d2[:, 0:32], avgT[0:NH, 0:32], Square, bias=avg[:, 0:1]
            ).then_inc(adone, 1)
            scalar.activation(
                d2[:, 32:64], avgT[0:NH, 32:64], Square, bias=avg[:, 1:2]
            ).then_inc(adone, 1)
            scalar.wait_ge(adone, 4)
            scalar.activation(
                pm[:], d2[:], Exp, scale=-0.5
            ).then_inc(a_exp, 1)

        @block.tensor
        def _(tensor):
            for c in range(NC):
                tensor.wait_ge(vdone, M[f"T{c}"])
                mm = tensor.matmul(
                    ps_st[:],
                    lhsT=EST[:, c * NH : (c + 1) * NH],
                    rhs=kf[:, 4 * c : 4 * c + 4],
                    start=(c == 0),
                    stop=(c == NC - 1),
                )
            mm.then_inc(p_st, 1)
            tensor.wait_ge(vdone, M["push"])
            tensor.matmul(
                ps_f[:], lhsT=ones[:], rhs=Z[:], start=True, stop=True
            ).then_inc(p_f, 1)

    nc.finalize()
    return nc


def _get_nc(NC, n_last):
    key = (NC, n_last)
    if key not in _CACHE:
        _CACHE[key] = _build_nc(NC, n_last)
    return _CACHE[key]


def kernel(tag_maps0, tag_maps1, kps0, kps1):
    from concourse.bass_utils import run_bass_kernel_spmd

    in_maps, NC, n_last = make_in_maps(tag_maps0, tag_maps1, kps0, kps1)
    nc = _get_nc(NC, n_last)
    out = run_bass_kernel_spmd(nc, in_maps, core_ids=list(range(B)))
    return np.array(
        [np.asarray(out.results[b]["out"]).reshape(()) for b in range(B)],
        dtype=np.float32,
    )


# revision 29
# speedup vs baseline: 1.0285x; 1.0207x over previous
"""Distributed Trainium2 Bass kernel for the associative-embedding (AE) loss.

Problem: per image b (B=8), two tag maps (tm0 [J,256,256], tm1 [J,512,512]),
keypoints kps [NH, 3*J] (x, y, vis interleaved, NH=30 humans, J=17 joints).
Per level: gather tag values at (j, x, y), masked per-human mean, pull loss
(masked squared deviation / num_humans) + push loss (pairwise Gaussian of
means / num_humans^2).  Output: per-image loss [B] (sum over both levels).

Strategy: pure data-parallel over B across 8 NeuronCores (core b handles
image b).  The loss touches only the ~NH*2*J visible-keypoint elements of
the 22 MB of tag maps, so each core pulls exactly those scalars out of DRAM
via indirect (SWDGE) DMAs.  The HW indirect DMA emits one descriptor per
out-partition row (max 128 scattered elements per ~1.1 us instruction), so
the host packs ONLY the visible entries into ceil(V/128) chunks -- typically
5 instead of 8 for the full grid -- with the small remainder chunk LAST so
its data drains quickly after the final descriptor-generation burst.  Host-
baked one-hot matrices let the tensor engine reduce the chunk layout into
per-human sufficient statistics via lhsT = E*S, rhs = [L0, L1, S*L0, S*L1]
(the two DVE prep ops per chunk are mutually independent, so the last
chunk's critical path is one op deep); chunks are processed while later
gathers are still in flight.  The push loss uses a 32x32 DVE stream
transpose, Square-with-bias + one Exp on the scalar engine, a ones-vector
matmul, and a 2-op weighted reduce; all input-only quantities (masks,
reciprocal counts, 1/num_humans weights) are host-precomputed.  Per-core
output is one scalar; the host stacks the 8 scalars into the final [B]
vector.
"""

import numpy as np

B = 8
NH = 30
J = 17
H0 = W0 = 256
H1 = W1 = 512
N0 = J * H0 * W0
N1 = J * H1 * W1
NTOT = N0 + N1
BIG = 30.0                # pad rows -> exp(-(BIG+avg)^2/2) ~ 1e-170 ~ 0
                          # (kept small: ACT Square is a piecewise table and
                          # must stay accurate at BIG, unlike huge sentinels)

_CACHE = {}


# ---------------------------------------------------------------------------
# host-side input prep: valid-packed gather indices + one-hot reduction maps
# ---------------------------------------------------------------------------


def make_in_maps(tag_maps0, tag_maps1, kps0, kps1):
    tag_maps0 = np.asarray(tag_maps0, dtype=np.float32)
    tag_maps1 = np.asarray(tag_maps1, dtype=np.float32)
    kps0 = np.asarray(kps0, dtype=np.int64)
    kps1 = np.asarray(kps1, dtype=np.int64)
    jr = np.arange(J)[None, :]
    per_img = []
    nv_max = 0
    for b in range(B):
        xs0, ys0, vs0 = kps0[b, :, 0::3], kps0[b, :, 1::3], kps0[b, :, 2::3]
        xs1, ys1, vs1 = kps1[b, :, 0::3], kps1[b, :, 1::3], kps1[b, :, 2::3]
        idx_hlj = np.concatenate(
            [jr * (H0 * W0) + xs0 * W0 + ys0,
             N0 + jr * (H1 * W1) + xs1 * W1 + ys1], axis=1
        )  # [30, 34] flat index per (human, level*J+joint)
        mask = np.concatenate([vs0 != 0, vs1 != 0], axis=1)  # [30, 34] bool
        hh, cc = np.nonzero(mask)     # valid entries: human, level*J+joint
        per_img.append((idx_hlj, mask, hh, cc))
        nv_max = max(nv_max, len(hh))
    NC = -(-nv_max // 128)            # chunks of <=128 descriptors
    n_last = nv_max - 128 * (NC - 1)  # last (remainder) chunk size

    in_maps = []
    for b in range(B):
        idx_hlj, mask, hh, cc = per_img[b]
        nv = len(hh)
        idxc = np.zeros((128, NC), np.int32)
        E = np.zeros((128, NC * NH), np.float32)
        T = np.zeros((128, 4 * NC), np.float32)  # [L0, L1, 0, 0] per chunk
        r = np.arange(nv)
        ch, p = r // 128, r % 128
        lvl = (cc // J).astype(np.int64)
        idxc[p, ch] = idx_hlj[hh, cc]
        E[p, ch * NH + hh] = 1.0
        T[p, ch * 4 + lvl] = 1.0

        cnt = np.stack([mask[:, :J].sum(1), mask[:, J:].sum(1)], 1).astype(
            np.float32
        )
        has = (cnt > 0).astype(np.float32)
        rdh = has / np.maximum(cnt, 1.0)
        P = 1.0 / has.sum(0)          # [2] 1/num_humans per level
        # kf layout [128, 4*NC + NC*NH + 8]:
        #   cols 0:4NC            T/rhs region (host: L0,L1,0,0; DVE: S*L)
        #   cols 4NC:4NC+NC*NH    E one-hot
        #   next 2: -rdh | next 2: rdh | next 4 (row 0): w4
        kf = np.zeros((128, 4 * NC + NC * NH + 8), np.float32)
        kf[:, 0 : 4 * NC] = T
        kf[:, 4 * NC : 4 * NC + NC * NH] = E
        base = 4 * NC + NC * NH
        kf[0:NH, base : base + 2] = -rdh
        kf[0:NH, base + 2 : base + 4] = rdh
        kf[0, base + 4 : base + 8] = [P[0] ** 2, P[1] ** 2, P[0], P[1]]
        tm = np.concatenate(
            [tag_maps0[b].ravel(), tag_maps1[b].ravel()]
        ).reshape(NTOT, 1)
        in_maps.append({"tm": tm, "ki": idxc, "kf": kf})
    return in_maps, NC, n_last


# ---------------------------------------------------------------------------
# device kernel (raw Block bass: hand-placed semaphores, no TileContext)
# ---------------------------------------------------------------------------


def _build_nc(NC, n_last):
    from contextlib import ExitStack

    from concourse import bacc, mybir
    from concourse.bass import IndirectOffsetOnAxis

    f32 = mybir.dt.float32
    i32 = mybir.dt.int32
    Alu = mybir.AluOpType
    X = mybir.AxisListType.X
    Exp = mybir.ActivationFunctionType.Exp
    Square = mybir.ActivationFunctionType.Square
    KFW = 4 * NC + NC * NH + 8
    base = 4 * NC + NC * NH

    nc = bacc.Bacc()
    TM = nc.declare_dram_parameter("tm", [NTOT, 1], f32, isOutput=False)
    KI = nc.declare_dram_parameter("ki", [128, NC], i32, isOutput=False)
    KF = nc.declare_dram_parameter("kf", [128, KFW], f32, isOutput=False)
    OUT = nc.declare_dram_parameter("out", [1, 1], f32, isOutput=True)

    with ExitStack() as ctx:
        e = ctx.enter_context
        ki = e(nc.sbuf_tensor("ki_sb", [128, NC], i32))
        kf = e(nc.sbuf_tensor("kf_sb", [128, KFW], f32))
        S = e(nc.sbuf_tensor("S", [128, NC], f32))
        EST = e(nc.sbuf_tensor("EST", [128, NC * NH], f32))
        avg = e(nc.sbuf_tensor("avg", [NH, 2], f32))
        u = e(nc.sbuf_tensor("u", [NH, 2], f32))
        avgsrc = e(nc.sbuf_tensor("avgsrc", [32, 64], f32))
        avgT = e(nc.sbuf_tensor("avgT", [32, 64], f32))
        d2 = e(nc.sbuf_tensor("d2", [NH, 64], f32))
        pm = e(nc.sbuf_tensor("pm", [NH, 64], f32))
        Z = e(nc.sbuf_tensor("Z", [NH, 4], f32))
        ones = e(nc.sbuf_tensor("ones", [NH, 1], f32))
        warm = e(nc.sbuf_tensor("warm", [1, 2], f32))
        fin = e(nc.sbuf_tensor("fin", [1, 4], f32))
        res = e(nc.sbuf_tensor("res", [1, 1], f32))
        ps_st = e(nc.psum_tensor("ps_st", [NH, 4], f32))
        ps_f = e(nc.psum_tensor("ps_f", [1, 4], f32))

        d_ki = e(nc.semaphore("d_ki"))
        d_kf = e(nc.semaphore("d_kf"))
        gs = [e(nc.semaphore(f"gs{c}")) for c in range(NC)]
        vdone = e(nc.semaphore("vdone"))
        adone = e(nc.semaphore("adone"))
        a_exp = e(nc.semaphore("a_exp"))
        p_st = e(nc.semaphore("p_st"))
        p_f = e(nc.semaphore("p_f"))
        d_out = e(nc.semaphore("d_out"))

        block = e(nc.Block())
        M = {}

        @block.vector
        def _(vector):
            n = 0

            def op(r, key=None):
                nonlocal n
                r.then_inc(vdone, 1)
                n += 1
                if key:
                    M[key] = n
                return n

            def wt(k):
                vector.wait_ge(vdone, k)

            # S zeroed so the partial last chunk's unwritten rows can't
            # inject NaNs through the 0-masked products
            op(vector.memset(S[:], 0.0), "sz")
            op(vector.memset(avgsrc[:], BIG))
            op(vector.memset(ones[:], 1.0))
            op(vector.memset(warm[:], 0.0), "warm")
            vector.wait_ge(d_kf, 16)
            # per chunk: rhs cols 2:4 = S*[L0,L1]; lhsT = E*S (independent)
            for c in range(NC):
                vector.wait_ge(gs[c], 16)
                op(vector.tensor_tensor(
                    out=kf[:, 4 * c + 2 : 4 * c + 4].rearrange(
                        "p (o l) -> p o l", o=1),
                    in0=S[:, c : c + 1].to_broadcast([128, 1, 2]),
                    in1=kf[:, 4 * c : 4 * c + 2].rearrange(
                        "p (o l) -> p o l", o=1),
                    op=Alu.mult))
                op(vector.tensor_tensor(
                    out=EST[:, c * NH : (c + 1) * NH].rearrange(
                        "p (o h) -> p o h", o=1),
                    in0=S[:, c : c + 1].to_broadcast([128, 1, NH]),
                    in1=kf[:, 4 * NC + c * NH : 4 * NC + (c + 1) * NH
                           ].rearrange("p (o h) -> p o h", o=1),
                    op=Alu.mult), f"T{c}")
            # stats landed in psum: bcast+transpose drive the push tail;
            # avg (the ACT bias) comes right after, off the transpose path
            vector.wait_ge(p_st, 1)
            a = op(vector.tensor_tensor(
                out=avgsrc[0:NH, :].rearrange("p (l j) -> p l j", l=2),
                in0=ps_st[:, 0:2].to_broadcast([NH, 2, 32]),
                in1=kf[0:NH, base : base + 2].to_broadcast([NH, 2, 32]),
                op=Alu.mult))
            wt(a)
            op(vector.transpose(avgT[:], avgsrc[:]))
            op(vector.tensor_tensor(
                out=avg[:], in0=ps_st[:, 0:2],
                in1=kf[0:NH, base + 2 : base + 4], op=Alu.mult), "avg")
            # pull stats while ACT runs the push exp
            wt(M["avg"])
            u_n = op(vector.tensor_tensor(
                out=u[:], in0=ps_st[:, 0:2], in1=avg[:], op=Alu.mult))
            wt(u_n)
            op(vector.tensor_tensor(
                out=Z[:, 2:4], in0=ps_st[:, 2:4], in1=u[:],
                op=Alu.subtract), "pull")
            # push row block sums once ACT finished the exp
            vector.wait_ge(a_exp, 1)
            op(vector.reduce_sum(
                out=Z[:, 0:2],
                in_=pm[:].rearrange("p (l j) -> p l j", l=2), axis=X),
                "push")
            # final weighted reduce of [push0, push1, pull0, pull1]
            vector.wait_ge(p_f, 1)
            a = op(vector.tensor_tensor(
                out=fin[:], in0=ps_f[:], in1=kf[0:1, base + 4 : base + 8],
                op=Alu.mult))
            wt(a)
            op(vector.reduce_sum(out=res[:], in_=fin[:], axis=X), "res")

        @block.sync
        def _(sync):
            # ki alone on the sync HWDGE queues so the big kf transfer
            # (issued by scalar) can't delay its completion semaphore
            sync.dma_start(out=ki[:], in_=KI[:]).then_inc(d_ki, 16)
            sync.wait_ge(vdone, M["res"])
            sync.dma_start(out=OUT[:], in_=res[:]).then_inc(d_out, 16)
            sync.wait_ge(d_out, 16)

        @block.gpsimd
        def _(gpsimd):
            gpsimd.wait_ge(d_ki, 16)
            gpsimd.wait_ge(vdone, M["sz"])
            for c in range(NC):
                rows = 128 if c < NC - 1 else n_last
                gpsimd.indirect_dma_start(
                    out=S[0:rows, c : c + 1],
                    out_offset=None,
                    in_=TM[:],
                    in_offset=IndirectOffsetOnAxis(
                        ap=ki[0:rows, c : c + 1], axis=0
                    ),
                ).then_inc(gs[c], 16)

        @block.scalar
        def _(scalar):
            scalar.dma_start(out=kf[:], in_=KF[:]).then_inc(d_kf, 16)
            scalar.wait_ge(vdone, M["warm"])
            scalar.activation(warm[:, 0:1], warm[:, 0:1], Exp).then_inc(
                adone, 1
            )
            scalar.activation(warm[:, 1:2], warm[:, 1:2], Square).then_inc(
                adone, 1
            )
            scalar.wait_ge(vdone, M["avg"])
            # d2[i, l*32+j] = (avg_j - avg_i)^2 ; avgT holds -avg_j, bias +avg_i
            scalar.activation(
                d2[:, 0:32], avgT[0:NH, 0:32], Square, bias=avg[:, 0:1]
            ).then_inc(adone, 1)
            scalar.activation(
                d2[:, 32:64], avgT[0:NH, 32:64], Square, bias=avg[:, 1:2]
            ).then_inc(adone, 1)
            scalar.wait_ge(adone, 4)
            scalar.activation(
                pm[:], d2[:], Exp, scale=-0.5
            ).then_inc(a_exp, 1)

        @block.tensor
        def _(tensor):
            for c in range(NC):
                tensor.wait_ge(vdone, M[f"T{c}"])
                mm = tensor.matmul(
                    ps_st[:],
                    lhsT=EST[:, c * NH : (c + 1) * NH],
                    rhs=kf[:, 4 * c : 4 * c + 4],
                    start=(c == 0),
                    stop=(c == NC - 1),
                )
            mm.then_inc(p_st, 1)
            tensor.wait_ge(vdone, M["push"])
            tensor.matmul(
                ps_f[:], lhsT=ones[:], rhs=Z[:], start=True, stop=True
            ).then_inc(p_f, 1)

    nc.finalize()
    return nc


def _get_nc(NC, n_last):
    key = (NC, n_last)
    if key not in _CACHE:
        _CACHE[key] = _build_nc(NC, n_last)
    return _CACHE[key]


def kernel(tag_maps0, tag_maps1, kps0, kps1):
    from concourse.bass_utils import run_bass_kernel_spmd

    in_maps, NC, n_last = make_in_maps(tag_maps0, tag_maps1, kps0, kps1)
    nc = _get_nc(NC, n_last)
    out = run_bass_kernel_spmd(nc, in_maps, core_ids=list(range(B)))
    return np.array(
        [np.asarray(out.results[b]["out"]).reshape(()) for b in range(B)],
        dtype=np.float32,
    )
